# revision 1
# baseline (speedup 1.0000x reference)
"""JointMLPDecoder TRN2 kernel: per-joint LayerNorm + MLP (D=512 -> 2048 -> 3).

Sharding: 24 joints split 3-per-core across 8 NeuronCores (expert-style).
Host packs x as x^T [J, D, B] so each core streams [d, b] tiles directly.

Per-core pipeline (joints j=0..2, batch chunks of 512):
  stats:  mean / E[x^2] over d via ones(1/512)-matmul on PE (broadcast rows)
  norm:   xn = (x - mu) * rstd  (2 DVE ops over [128, 4*512] tiles, in-place)
  gemm1:  H^T[m-chunk, b] = sum_d w1[d, m].T @ xn[d, b]   (float32r)
  gelu:   h = Gelu(H^T + b1)   (single ACT op, PSUM -> SBUF)
  gemm2:  y^T[3, b] = sum_m w2[m-chunk, 3].T @ h           (float32r)
  out:    y^T + b2 -> DRAM [3, 3, B] per core; host transposes to [B, 1, 24, 3]

LayerNorm affine (ln_g, ln_b) is folded into w1/b1 on the host:
  (xn*g + b) @ w1 + b1 == xn @ (g*w1) + (b1 + b @ w1)
"""

import numpy as np
from contextlib import ExitStack

import concourse.bass as bass
import concourse.bacc as bacc
import concourse.tile as tile
from concourse import mybir
from concourse import bass_utils

F32 = mybir.dt.float32
F32R = mybir.dt.float32r
AF = mybir.ActivationFunctionType
ALU = mybir.AluOpType

B = 4096
J = 24
D = 512
M = 2048
NCORES = 8
JPC = J // NCORES          # 3 joints per core
BCH = 512                  # batch chunk (matmul N)
NBC = B // BCH             # 8
NDC = D // 128             # 4 contraction chunks for gemm1
NMC = M // 128             # 16 contraction chunks for gemm2
EPS = 1e-5

_CACHE: dict = {}


def _bcast_dc(t, n):
    """View a [128, BCH] tile/AP as [128, n, BCH] with stride-0 middle dim."""
    ap = t[:, :]
    new_ap = [list(ap.ap[0]), [0, n], list(ap.ap[-1])]
    return bass.AP(tensor=ap.tensor, offset=ap.offset, ap=new_ap)


def build_body(nc, tc, ctx, jpc=JPC, nbc=NBC, gelu=True):
    xT = nc.dram_tensor("xT", [jpc, D, B], F32R, kind="ExternalInput").ap()
    w1 = nc.dram_tensor("w1", [jpc, D, M], F32R, kind="ExternalInput").ap()
    b1 = nc.dram_tensor("b1", [jpc, 128, NMC], F32, kind="ExternalInput").ap()
    w2 = nc.dram_tensor("w2", [jpc, 128, NMC, 3], F32R, kind="ExternalInput").ap()
    b2 = nc.dram_tensor("b2", [jpc, 3, 1], F32, kind="ExternalInput").ap()
    ones = nc.dram_tensor("ones", [128, 128], F32R, kind="ExternalInput").ap()
    yT = nc.dram_tensor("yT", [jpc, 3, B], F32, kind="ExternalOutput").ap()

    consts = ctx.enter_context(tc.tile_pool(name="consts", bufs=1))
    wpool = ctx.enter_context(tc.tile_pool(name="wpool", bufs=2))
    xpool = ctx.enter_context(tc.tile_pool(name="xpool", bufs=2))
    spool = ctx.enter_context(tc.tile_pool(name="spool", bufs=2))
    hpool = ctx.enter_context(tc.tile_pool(name="hpool", bufs=4))
    opool = ctx.enter_context(tc.tile_pool(name="opool", bufs=2))
    ps_stats = ctx.enter_context(tc.tile_pool(name="ps_stats", bufs=1, space="PSUM"))
    ps_g1 = ctx.enter_context(tc.tile_pool(name="ps_g1", bufs=4, space="PSUM"))
    ps_g2 = ctx.enter_context(tc.tile_pool(name="ps_g2", bufs=2, space="PSUM"))

    ones_t = consts.tile([128, 128], F32R)
    nc.sync.dma_start(out=ones_t, in_=ones)
    eps_t = consts.tile([128, 1], F32)
    nc.vector.memset(eps_t, EPS)

    jw = {}  # per-joint weight tiles, keyed by j

    def emit_stats(j, bc, var2, idx):
        """Phase A for iteration (j, bc): weight DMA (at j start), x load,
        stats matmuls, and the full normalize chain producing xn."""
        if bc == 0:
            w1_t = [wpool.tile([128, M], F32R, name=f"w1_{dc}", tag=f"w1_{dc}")
                    for dc in range(NDC)]
            for dc in range(NDC):
                nc.sync.dma_start(out=w1_t[dc],
                                  in_=w1[j, dc * 128:(dc + 1) * 128, :])
            w2_t = wpool.tile([128, NMC, 3], F32R, name="w2_t", tag="w2_t")
            nc.sync.dma_start(out=w2_t, in_=w2[j])
            b1_t = wpool.tile([128, NMC], F32, name="b1_t", tag="b1_t")
            nc.sync.dma_start(out=b1_t, in_=b1[j])
            b2_t = wpool.tile([3, 1], F32, name="b2_t", tag="b2_t")
            nc.sync.dma_start(out=b2_t, in_=b2[j])
            jw[j] = (w1_t, w2_t, b1_t, b2_t)

        bsl = slice(bc * BCH, (bc + 1) * BCH)
        xt = xpool.tile([128, NDC, BCH], F32R, name="xt", tag="xt")
        nc.sync.dma_start(
            out=xt,
            in_=xT[j, :, bsl].rearrange("(dc p) b -> p dc b", p=128),
        )
        xt_f = xt.rearrange("p dc b -> p (dc b)").bitcast(F32)

        xsq = xpool.tile([128, NDC, BCH], F32R, name="xsq", tag="xsq", bufs=1)
        nc.vector.tensor_mul(xsq.rearrange("p dc b -> p (dc b)"), xt_f, xt_f)

        ps_mu = ps_stats.tile([128, BCH], F32, name="ps_mu", tag="ps_mu")
        ps_ms = ps_stats.tile([128, BCH], F32, name="ps_ms", tag="ps_ms")
        for dc in range(NDC):
            nc.tensor.matmul(ps_mu, ones_t, xt[:, dc, :],
                             start=(dc == 0), stop=(dc == NDC - 1))
        for dc in range(NDC):
            nc.tensor.matmul(ps_ms, ones_t, xsq[:, dc, :],
                             start=(dc == 0), stop=(dc == NDC - 1))

        mu_t = spool.tile([128, BCH], F32, name="mu_t", tag="mu_t")
        nc.vector.tensor_copy(mu_t, ps_mu)
        # var = E[x^2] - mu^2, written into this pair's half of var2
        msq_t = spool.tile([128, BCH], F32, name="msq_t", tag="msq_t")
        nc.vector.scalar_tensor_tensor(
            out=msq_t, in0=mu_t, scalar=-1.0, in1=mu_t,
            op0=ALU.mult, op1=ALU.mult)
        nc.vector.tensor_add(var2[:, idx, :], msq_t, ps_ms)

        # center: xn = x - mu  (rstd applied after the pair sqrt)
        mu_b = _bcast_dc(mu_t, NDC)
        xn = xpool.tile([128, NDC, BCH], F32R, name="xn", tag="xn", bufs=4)
        nc.vector.tensor_sub(xn, xt.bitcast(F32), mu_b)
        return (j, bc, xn, var2, idx)

    def finish_pair(states):
        """One sqrt+reciprocal over the pair's two var tiles (one ACT table
        swap per pair instead of per iteration), then apply rstd."""
        var2 = states[0][3]
        n = len(states)
        v_flat = var2.rearrange("p i b -> p (i b)")[:, :n * BCH]
        std2 = spool.tile([128, 2, BCH], F32, name="std2", tag="std2")
        nc.scalar.activation(std2.rearrange("p i b -> p (i b)")[:, :n * BCH],
                             v_flat, AF.Sqrt, bias=eps_t, scale=1.0)
        nc.vector.reciprocal(std2.rearrange("p i b -> p (i b)")[:, :n * BCH],
                             std2.rearrange("p i b -> p (i b)")[:, :n * BCH])
        for (_, _, xn, _, idx) in states:
            rstd_b = _bcast_dc(std2[:, idx, :], NDC)
            nc.vector.tensor_mul(xn, xn.bitcast(F32), rstd_b)

    def emit_gemms(state):
        """Phase B for iteration (j, bc): gemm1 + gelu + gemm2 + output."""
        j, bc, xn = state[0], state[1], state[2]
        w1_t, w2_t, b1_t, b2_t = jw[j]
        bsl = slice(bc * BCH, (bc + 1) * BCH)

        ps_y = ps_g2.tile([3, BCH], F32, name="ps_y", tag="ps_y")
        for mc in range(NMC):
            ps_h = ps_g1.tile([128, BCH], F32, name="ps_h", tag="ps_h")
            msl = slice(mc * 128, (mc + 1) * 128)
            for dc in range(NDC):
                nc.tensor.matmul(ps_h, w1_t[dc][:, msl], xn[:, dc, :],
                                 start=(dc == 0), stop=(dc == NDC - 1))
            h_t = hpool.tile([128, BCH], F32R, name="h_t", tag="h_t")
            nc.scalar.activation(h_t, ps_h,
                                 AF.Gelu if gelu else AF.Identity,
                                 bias=b1_t[:, mc:mc + 1], scale=1.0)
            nc.tensor.matmul(ps_y, w2_t[:, mc, :], h_t,
                             start=(mc == 0), stop=(mc == NMC - 1))

        y_sb = opool.tile([3, BCH], F32, name="y_sb", tag="y_sb")
        nc.vector.tensor_scalar_add(y_sb, ps_y, b2_t)
        nc.sync.dma_start(out=yT[j, :, bsl], in_=y_sb)

    # depth-2 software pipeline over iteration pairs: stats of pair p+1 are
    # emitted ahead of the gemm phases of pair p; each pair shares one
    # sqrt+reciprocal (one ACT table swap per pair instead of per iteration)
    iters = [(j, bc) for j in range(jpc) for bc in range(nbc)]
    prev_states = None
    for i in range(0, len(iters), 2):
        chunk = iters[i:i + 2]
        var2 = spool.tile([128, 2, BCH], F32, name="var2", tag="var2")
        states = [emit_stats(j, bc, var2, k) for k, (j, bc) in enumerate(chunk)]
        finish_pair(states)
        if prev_states is not None:
            for st in prev_states:
                emit_gemms(st)
        prev_states = states
    for st in prev_states:
        emit_gemms(st)


def _build_nc(jpc=JPC, nbc=NBC, reps=1, gelu=True):
    nc = bacc.Bacc("TRN2", target_bir_lowering=False, debug=False, num_devices=NCORES)
    with tile.TileContext(nc) as tc, ExitStack() as ctx:
        if reps == 1:
            build_body(nc, tc, ctx, jpc, nbc, gelu)
        else:
            # timing variant: repeat the whole body in a hardware loop
            def body(_i, unroll=1):
                with ExitStack() as c2:
                    build_body(nc, tc, c2, jpc, nbc, gelu)
            with tc.For_i(0, reps, 1) as i:
                body(i)
    nc.compile()
    return nc


def _pack_inputs(x, ln_g, ln_b, w1, b1, w2, b2):
    x = np.asarray(x)
    w1 = np.asarray(w1)
    b1 = np.asarray(b1)
    w2 = np.asarray(w2)
    b2 = np.asarray(b2)
    ln_g = np.asarray(ln_g)
    ln_b = np.asarray(ln_b)

    # fold LN affine into w1/b1
    if not (ln_g == 1.0).all():
        w1 = ln_g[:, :, None] * w1
    if not (ln_b == 0.0).all():
        b1 = b1 + np.einsum("jd,jdm->jm", ln_b, w1)

    xT = np.ascontiguousarray(x.transpose(1, 2, 0))          # [J, D, B]
    w2p = np.ascontiguousarray(
        w2.reshape(J, NMC, 128, 3).transpose(0, 2, 1, 3))    # [J, 128, NMC, 3]
    b1p = np.ascontiguousarray(
        b1.reshape(J, NMC, 128).transpose(0, 2, 1))          # [J, 128, NMC]
    b2p = np.ascontiguousarray(b2.reshape(J, 3, 1))
    ones = np.full((128, 128), 1.0 / D, dtype=np.float32)

    in_maps = []
    for c in range(NCORES):
        js = slice(c * JPC, (c + 1) * JPC)
        in_maps.append({
            "xT": xT[js],
            "w1": np.ascontiguousarray(w1[js]),
            "b1": b1p[js],
            "w2": w2p[js],
            "b2": b2p[js],
            "ones": ones,
        })
    return in_maps


def kernel(x, ln_g, ln_b, w1, b1, w2, b2):
    if "nc" not in _CACHE:
        _CACHE["nc"] = _build_nc()
    nc = _CACHE["nc"]

    in_maps = _pack_inputs(x, ln_g, ln_b, w1, b1, w2, b2)
    res = bass_utils.run_bass_kernel_spmd(nc, in_maps, core_ids=list(range(NCORES)))

    # yT per core: [JPC, 3, B] -> y [B, 1, J, 3]
    yT = np.stack([res.results[c]["yT"] for c in range(NCORES)])  # [8, JPC, 3, B]
    y = yT.reshape(J, 3, B).transpose(2, 0, 1)[:, None, :, :]
    return np.ascontiguousarray(y.astype(np.float32))

